# revision 32
# baseline (speedup 1.0000x reference)
"""CRF forward-algorithm kernel for Trainium2 (8 NeuronCores, Bass/Tile).

Problem: emissions [128, 512, 256] f32, mask [128, 512] bool,
start/end_transitions [256], transitions [256, 256].
reference = partition - score where both are logsumexp forward scans over
seq_len; score applies the mask at each step, partition does not.

Strategy
--------
Data-parallel over batch: 16 batch rows per core on 8 cores; the scan over
seq_len stays local per device (per the sharding hint).

Per-device math runs the *scaled forward algorithm* in linear space, split
bidirectionally to halve the sequential-dependency depth: a forward chain
    alpha_t[j, b] = (sum_i E1[i, j] * alpha_{t-1}[i, b]) * W1_t[j, b]
from t=1..TM and a backward chain
    delta_t[i, b] = (sum_j E2[j, i] * delta_{t+1}[j, b]) * W2_t[i, b]
from t=S-2..TM+1 run concurrently (independent per-step latency chains that
the Tile scheduler interleaves on the PE/DVE engines), meeting at
    Z[b] = sum_i alpha_TM[i,b] * e^{c2[i]} * (E2^T delta_{TM+1})[i,b].
E1 = exp(Tr - colmax), E2 = exp(Tr^T - colmax(Tr^T)) are constant stationary
bf16 weights (4 chunks each); W1/W2 = exp(em + c) are streamed from HBM.
Every NORM_EVERY steps each chain rescales per batch row (divide by tag 0's
value — textbook scaling) so f32/bf16 never overflows; scale factors cancel
between the partition and score logsumexp terms.

Tag dim T=256 sits on SBUF partitions packed [128 partitions, 2 halves x 16
batch]; each chain step is 4 matmuls (2 K-chunks x 2 M-chunks, fp32 PSUM
accumulate) + 2 vector multiplies (PSUM x W -> next bf16 state, split by
output half so DVE overlaps PE).

With the all-ones mask of this problem the masked (score) and unmasked
(partition) scans are identical computations, so the shared scan is computed
once; score and partition are then two identical log reductions of the same
Z and the device returns their difference (exactly 0.0, bitwise-matching the
reference, which also computes two identical scans).  A general-mask numpy
fallback handles any other mask.

Sync-topology note: the TensorTensor/Matmult ISA slots fit a single sync
wait, so the hot loop is arranged so every instruction needs at most one
un-observed semaphore tick: state tiles get a unique tag per step (no WAW),
and each W chunk's DMA wait is absorbed by a tiny DVE "probe" copy ordered
before the chunk's first TT (Bacc's legalizer splits any remainder).
"""

import numpy as np

B, S, T = 128, 512, 256
NCORES = 8
BL = B // NCORES  # 16 batch rows per core
TH = T // 2  # 128: tags per partition-half
PACK = 2 * BL  # 32: packed free dim = [half, batch]
NORM_EVERY = 128
CHUNK = 64  # scan steps per W DMA chunk
CHUNK0 = 16  # first chunk small so compute starts early
WDECAY = -9  # W is pre-scaled by 2^WDECAY on host so the state stays ~flat
# (mean per-step growth is ~e^6.3 ~= 2^9.1); per-batch random-walk drift over
# a NORM_EVERY window is verified well inside f32 range by the host bit-sim.
TM = (S - 1) // 2  # forward chain steps (255); backward gets S-2-TM (255)

_NC_CACHE = {}


def _chunks(n, chunk, chunk0):
    """Split n steps into DMA chunk sizes: one small first chunk so compute
    starts early, then uniform big chunks (>=4KB per-partition descriptor
    runs keep the DMA near peak bandwidth)."""
    sizes = []
    if n > chunk0:
        sizes.append(chunk0)
        n -= chunk0
    while n > 0:
        c = min(chunk, n)
        sizes.append(c)
        n -= c
    return sizes


def _build_nc(
    seqlen=S, norm_every=NORM_EVERY, chunk=CHUNK, chunk0=CHUNK0, debug_alpha=True
):
    """Build the Bass/Tile program (shared SPMD NEFF for all 8 cores)."""
    import concourse.tile as tile
    from concourse import bacc, mybir
    from concourse.tile_rust import add_dep_helper

    f32 = mybir.dt.float32
    bf16 = mybir.dt.bfloat16
    Alu = mybir.AluOpType
    Act = mybir.ActivationFunctionType

    nsteps = seqlen - 1  # total matmul phases (incl. combine)
    tm = (nsteps - 1) // 2  # forward steps
    nb = nsteps - 1 - tm  # backward steps

    # Bacc (not raw Bass): its compile pipeline legalizes sync waits
    # (1-wait-per-instruction ISA limit) and moves matmul waits to ldweights.
    nc = bacc.Bacc("TRN2", target_bir_lowering=False)
    win1 = nc.declare_dram_parameter("win1", [TH, max(tm, 1), PACK], bf16, isOutput=False)
    win2 = nc.declare_dram_parameter("win2", [TH, max(nb, 1), PACK], bf16, isOutput=False)
    p0d = nc.declare_dram_parameter("p0", [TH, PACK], bf16, isOutput=False)
    d0d = nc.declare_dram_parameter("d0", [TH, PACK], bf16, isOutput=False)
    econd = nc.declare_dram_parameter("econ", [TH, 8, TH], bf16, isOutput=False)
    cmbd = nc.declare_dram_parameter("cmb", [TH, PACK], f32, isOutput=False)
    outd = nc.declare_dram_parameter("out", [1, BL], f32, isOutput=True)
    if debug_alpha:
        alphad = nc.declare_dram_parameter("alpha", [TH, PACK], f32, isOutput=True)
        betad = nc.declare_dram_parameter("beta", [TH, PACK], f32, isOutput=True)

    sizes1 = _chunks(tm, chunk, chunk0)
    sizes2 = _chunks(nb, chunk, chunk0)

    with tile.TileContext(nc) as tc:
        from contextlib import ExitStack

        with ExitStack() as ctx:
            const = ctx.enter_context(tc.tile_pool(name="const", bufs=1))
            wpool = ctx.enter_context(tc.tile_pool(name="wpool", bufs=1))
            probes = ctx.enter_context(tc.tile_pool(name="probes", bufs=1))
            ppool = ctx.enter_context(tc.tile_pool(name="ppool", bufs=1))
            mpool = ctx.enter_context(tc.tile_pool(name="mpool", bufs=1, space="PSUM"))
            bcpool = ctx.enter_context(tc.tile_pool(name="bcpool", bufs=1, space="PSUM"))
            spool = ctx.enter_context(tc.tile_pool(name="spool", bufs=1, space="PSUM"))
            stage = ctx.enter_context(tc.tile_pool(name="stage", bufs=4))
            fin = ctx.enter_context(tc.tile_pool(name="fin", bufs=1))

            # Constants
            e_t = const.tile([TH, 8, TH], bf16, tag="e_t")
            nc.sync.dma_start(out=e_t[:], in_=econd[:])
            cmb_t = const.tile([TH, PACK], f32, tag="cmb_t")
            nc.sync.dma_start(out=cmb_t[:], in_=cmbd[:])
            ones_row = const.tile([1, TH], f32, tag="ones_row")
            nc.vector.memset(ones_row[:], 1.0)
            ones_col = const.tile([TH, 1], f32, tag="ones_col")
            nc.vector.memset(ones_col[:], 1.0)

            # Probe absorbing the cmb DMA wait onto the DVE clock.
            cmb_probe = probes.tile([1, 1], f32, tag="cmb_probe")
            cmb_probe_inst = nc.vector.tensor_copy(cmb_probe[:], cmb_t[0:1, 0:1])

            # Stream all W chunks up front (independent tiles; compute on a
            # chunk starts as soon as its DMA lands).  The two chains' DMAs
            # are interleaved in issue order so the backward chain's early
            # chunks aren't queued behind the whole forward stream.
            def alloc_w(sizes, name):
                tiles = []
                t0 = 0
                for k, n in enumerate(sizes):
                    wt = wpool.tile([TH, n, PACK], bf16, tag=f"{name}_{k}")
                    tiles.append((t0, n, wt))
                    t0 += n
                return tiles

            pf_init = ppool.tile([TH, PACK], bf16, tag="pf_init")
            nc.sync.dma_start(out=pf_init[:], in_=p0d[:])
            pb_init = ppool.tile([TH, PACK], bf16, tag="pb_init")
            nc.sync.dma_start(out=pb_init[:], in_=d0d[:])
            p_cur = [pf_init, pb_init]

            wts1 = alloc_w(sizes1, "w1")
            wts2 = alloc_w(sizes2, "w2")
            # Issue only the first chunk of each chain up front; the bulk
            # chunk DMAs are deferred into the step loop so their (serialized,
            # ~1us each) SP-sequencer issue overlaps with early compute.
            deferred_dmas = []
            for k in range(max(len(wts1), len(wts2))):
                for dram, wts in ((win1, wts1), (win2, wts2)):
                    if k < len(wts):
                        t0, n, wt = wts[k]
                        if k == 0:
                            nc.sync.dma_start(out=wt[:], in_=dram[:, t0 : t0 + n, :])
                        else:
                            deferred_dmas.append((dram, t0, n, wt))

            chain_w = [wts1, wts2]
            chain_nsteps = [tm, nb]
            chain_ci = [0, 0]  # current chunk index per chain

            def emit_step(ch, t):
                """One recurrence step for chain ch (0=fwd, 1=bwd) at local
                step t: 4 matmuls into 2 PSUM halves + 2 TT multiplies."""
                ci = chain_ci[ch]
                t0, n, wt = chain_w[ch][ci]
                if t == t0:
                    probe = probes.tile([1, 1], bf16, tag=f"probe{ch}_{ci}")
                    probe_inst = nc.vector.tensor_copy(probe[:], wt[0:1, 0:1, 0:1])
                else:
                    probe_inst = None
                off = t - t0
                p_prev = p_cur[ch]
                pnew = ppool.tile([TH, PACK], bf16, tag=f"p{ch}_{t}")
                for q in (0, 1):
                    mm = mpool.tile([TH, BL], f32, tag=f"mm{ch}q{q}")
                    for h in (0, 1):
                        nc.tensor.matmul(
                            mm[:],
                            lhsT=e_t[:, ch * 4 + h * 2 + q, :],
                            rhs=p_prev[:, h * BL : (h + 1) * BL],
                            start=(h == 0),
                            stop=(h == 1),
                        )
                    tt = nc.vector.tensor_tensor(
                        pnew[:, q * BL : (q + 1) * BL],
                        mm[:],
                        wt[:, off, q * BL : (q + 1) * BL],
                        Alu.mult,
                    )
                    if probe_inst is not None:
                        add_dep_helper(tt.ins, probe_inst.ins, False)
                if t == t0 + n - 1:
                    chain_ci[ch] += 1
                # Norm every NORM_EVERY steps AND on the chain's last step:
                # the combine multiplies alpha*N, so both factors must be
                # freshly rescaled or their product overflows f32.  The two
                # chains' norm phases are staggered so their extra critical-
                # path latency doesn't land on the same tick.
                phase = norm_every - 1 if ch == 0 else norm_every // 2 - 1
                if (t % norm_every) == phase or t == chain_nsteps[ch] - 1:
                    # Rescale each batch row by 1 / P[tag 0, b] (same factor
                    # for every tag of a given b -> pure per-b scaling).
                    # reciprocal -> rr -> bc matmuls -> pn2 is a data chain,
                    # so the reciprocal (which carries the DVE wait for pnew)
                    # always precedes pn2 on the DVE queue.
                    rr = stage.tile([1, PACK], f32, tag=f"rr{ch}")
                    nc.vector.reciprocal(rr[0:1, 0:BL], pnew[0:1, 0:BL])
                    nc.vector.tensor_copy(rr[0:1, BL:PACK], rr[0:1, 0:BL])
                    bc = bcpool.tile([TH, PACK], f32, tag=f"bc{ch}")
                    nc.tensor.matmul(
                        bc[:], lhsT=ones_row[0:1, :], rhs=rr[:], start=True, stop=True
                    )
                    pn2 = ppool.tile([TH, PACK], bf16, tag=f"p{ch}_{t}n")
                    nc.vector.tensor_tensor(pn2[:], pnew[:], bc[:], Alu.mult)
                    pnew = pn2
                p_cur[ch] = pnew

            # Interleave the two chains so the scheduler anti-phases them.
            for t in range(max(tm, nb)):
                if t < tm:
                    emit_step(0, t)
                if t < nb:
                    emit_step(1, t)
                if t == 8:
                    for dram, t0, n, wt in deferred_dmas:
                        nc.sync.dma_start(out=wt[:], in_=dram[:, t0 : t0 + n, :])

            # Combine: N = E2^T delta (4 matmuls), then
            # S[b] = sum_i alpha[i,b] * cmb[i] * N[i,b]; score/partition = ln S.
            alpha_f = p_cur[0]
            delta_b = p_cur[1]
            tmp1 = fin.tile([TH, PACK], f32, tag="tmp1")
            t1_tt = nc.vector.tensor_tensor(tmp1[:], alpha_f[:], cmb_t[:], Alu.mult)
            add_dep_helper(t1_tt.ins, cmb_probe_inst.ins, False)
            tmp2 = fin.tile([TH, PACK], f32, tag="tmp2")
            for q in (0, 1):
                mmn = mpool.tile([TH, BL], f32, tag=f"mm1q{q}")
                for h in (0, 1):
                    nc.tensor.matmul(
                        mmn[:],
                        lhsT=e_t[:, 4 + h * 2 + q, :],
                        rhs=delta_b[:, h * BL : (h + 1) * BL],
                        start=(h == 0),
                        stop=(h == 1),
                    )
                nc.vector.tensor_tensor(
                    tmp2[:, q * BL : (q + 1) * BL],
                    mmn[:],
                    tmp1[:, q * BL : (q + 1) * BL],
                    Alu.mult,
                )
            s_ps = spool.tile([1, PACK], f32, tag="s_ps")
            nc.tensor.matmul(
                s_ps[:], lhsT=ones_col[:, 0:1], rhs=tmp2[:], start=True, stop=True
            )
            s_sb = fin.tile([1, PACK], f32, tag="s_sb")
            nc.scalar.copy(s_sb[:], s_ps[:])
            stot = fin.tile([1, BL], f32, tag="stot")
            nc.vector.tensor_tensor(
                stot[:], s_sb[0:1, 0:BL], s_sb[0:1, BL:PACK], Alu.add
            )
            # score scan == partition scan under the all-ones mask; both
            # logsumexp terms are computed and subtracted.
            lg_score = fin.tile([1, BL], f32, tag="lg_score")
            nc.scalar.activation(lg_score[:], stot[:], Act.Ln)
            lg_part = fin.tile([1, BL], f32, tag="lg_part")
            nc.scalar.activation(lg_part[:], stot[:], Act.Ln)
            oo = fin.tile([1, BL], f32, tag="oo")
            nc.vector.tensor_tensor(oo[:], lg_part[:], lg_score[:], Alu.subtract)
            nc.sync.dma_start(out=outd[:], in_=oo[:])

            if debug_alpha:
                al = fin.tile([TH, PACK], f32, tag="al")
                nc.vector.tensor_copy(al[:], alpha_f[:])
                nc.sync.dma_start(out=alphad[:], in_=al[:])
                be = fin.tile([TH, PACK], f32, tag="be")
                nc.vector.tensor_copy(be[:], delta_b[:])
                nc.sync.dma_start(out=betad[:], in_=be[:])

    return nc


def _get_nc(**kw):
    key = tuple(sorted(kw.items()))
    if key not in _NC_CACHE:
        nc = _build_nc(**kw)
        nc.finalize()  # run the Bacc legalization/compile pipeline
        _NC_CACHE[key] = nc
    return _NC_CACHE[key]


def _pack(a):
    """[BL, T] per-batch-major -> packed [TH, 2*BL] = [tagmod, half*BL+b]."""
    return np.ascontiguousarray(
        a.T.reshape(2, TH, BL).transpose(1, 0, 2).reshape(TH, PACK)
    )


def prepare_inputs(emissions, start_transitions, transitions, end_transitions,
                   seqlen=S):
    """Host-side packing of the per-core Bass inputs (all numpy)."""
    import ml_dtypes

    bf16 = ml_dtypes.bfloat16
    em = np.asarray(emissions, dtype=np.float32)[:, :seqlen]
    st = np.asarray(start_transitions, dtype=np.float32)
    tr = np.asarray(transitions, dtype=np.float32)
    en = np.asarray(end_transitions, dtype=np.float32)

    nsteps = seqlen - 1
    tm = (nsteps - 1) // 2
    nb = nsteps - 1 - tm

    c1 = tr.max(axis=0)  # [T] col max
    E1 = np.exp(tr - c1[None, :])
    tr2 = np.ascontiguousarray(tr.T)
    c2 = tr2.max(axis=0)  # = row max of tr
    E2 = np.exp(tr2 - c2[None, :])

    # econ[kmod, chain*4 + h*2 + q, mcol] = E[h*128+kmod, q*128+mcol]
    def chunks4(E):
        return E.reshape(2, TH, 2, TH).transpose(1, 0, 2, 3).reshape(TH, 4, TH)

    econ = np.ascontiguousarray(
        np.concatenate([chunks4(E1), chunks4(E2)], axis=1)
    ).astype(bf16)

    # cmb[imod, h*16+b] = exp(c2[h*128+imod]) replicated over b
    cmb = np.ascontiguousarray(
        np.repeat(np.exp(c2).reshape(2, TH).T[:, :, None], BL, axis=2).reshape(
            TH, PACK
        )
    ).astype(np.float32)

    def pack_w(X):
        # X: [BL, n, T] -> [TH, n, PACK]
        n = X.shape[1]
        return np.ascontiguousarray(
            X.transpose(2, 1, 0)  # [T, n, BL]
            .reshape(2, TH, n, BL)  # [h, tagmod, t, b]
            .transpose(1, 2, 0, 3)  # [tagmod, t, h, b]
            .reshape(TH, n, PACK)
        ).astype(bf16)

    in_maps = []
    for k in range(NCORES):
        em_k = em[k * BL : (k + 1) * BL]  # [BL, seqlen, T]
        # forward init: alpha_0 = exp(start + em_0 - rowmax)
        u0 = st[None, :] + em_k[:, 0, :]
        p0 = np.exp(u0 - u0.max(axis=1, keepdims=True))
        # backward init: delta_{S-1} = exp(em_{S-1} + end - rowmax)
        v0 = en[None, :] + em_k[:, seqlen - 1, :]
        d0 = np.exp(v0 - v0.max(axis=1, keepdims=True))
        # forward W: steps t = 1..tm (2^WDECAY keeps the running state flat)
        W1 = np.ldexp(np.exp(em_k[:, 1 : tm + 1, :] + c1[None, None, :]), WDECAY)
        # backward W: execution order k=0..nb-1 maps to t = seqlen-2-k
        emb = em_k[:, seqlen - 2 : seqlen - 2 - nb : -1, :] if nb else em_k[:, :0, :]
        W2 = np.ldexp(np.exp(emb + c2[None, None, :]), WDECAY)
        in_maps.append(
            {
                "win1": pack_w(W1) if tm else np.zeros((TH, 1, PACK), bf16),
                "win2": pack_w(W2) if nb else np.zeros((TH, 1, PACK), bf16),
                "p0": _pack(p0).astype(bf16),
                "d0": _pack(d0).astype(bf16),
                "econ": econ,
                "cmb": cmb,
            }
        )
    return in_maps


def run_on_device(in_maps, trace=False, **build_kw):
    from concourse.bass_utils import run_bass_kernel_spmd

    nc = _get_nc(**build_kw)
    res = run_bass_kernel_spmd(nc, in_maps, list(range(NCORES)), trace=trace)
    return res


def _numpy_crf(em, mask, st, en, tr):
    """General-mask fallback mirroring the reference (log space, float32)."""

    def lse(x, axis):
        m = x.max(axis=axis, keepdims=True)
        return (m + np.log(np.exp(x - m).sum(axis=axis, keepdims=True))).squeeze(axis)

    init = st[None, :] + em[:, 0]  # [B, T]
    score = init.copy()
    alpha = init.copy()
    for t in range(1, em.shape[1]):
        inner_s = score[:, :, None] + tr[None, :, :] + em[:, t][:, None, :]
        nxt = lse(inner_s, 1)
        score = np.where(mask[:, t][:, None], nxt, score)
        inner_a = alpha[:, :, None] + tr[None, :, :] + em[:, t][:, None, :]
        alpha = lse(inner_a, 1)
    s = lse(score + en[None, :], 1)
    p = lse(alpha + en[None, :], 1)
    return (p - s).astype(np.float32)


def kernel(emissions, mask, start_transitions, end_transitions, transitions):
    em = np.asarray(emissions, dtype=np.float32)
    mk = np.asarray(mask).astype(bool)
    st = np.asarray(start_transitions, dtype=np.float32)
    en = np.asarray(end_transitions, dtype=np.float32)
    tr = np.asarray(transitions, dtype=np.float32)

    if not mk[:, 1:].all():
        # With step masking active the score scan differs from the partition
        # scan; handle that general case on host.
        return _numpy_crf(em, mk, st, en, tr)

    in_maps = prepare_inputs(em, st, tr, en)
    res = run_on_device(in_maps)
    out = np.concatenate(
        [np.asarray(res.results[k]["out"]).reshape(BL) for k in range(NCORES)]
    )
    return out.astype(np.float32)


if __name__ == "__main__":
    rng = np.random.default_rng(0)
    em = rng.standard_normal((B, S, T), dtype=np.float32)
    mk = np.ones((B, S), dtype=bool)
    st = rng.standard_normal(T).astype(np.float32)
    en = rng.standard_normal(T).astype(np.float32)
    tr = rng.standard_normal((T, T)).astype(np.float32)
    out = kernel(em, mk, st, en, tr)
    print("out", out.shape, out.dtype, "absmax", np.abs(out).max())
